# revision 1
# baseline (speedup 1.0000x reference)
"""GAT policy network (3-layer GAT + global mean pool head) on 8 Trainium2
NeuronCores via Bass/Tile.

Sharding: nodes are dealt to the 8 cores (graph/data parallel); each core owns
6250 dst nodes (padded to 6272 = 49 tiles x 128) and all edges incident on them
by destination.  Small GAT weights are replicated.

Core ideas:
  * Rectangular slot grid per core: dst nodes sorted by in-degree, partition =
    dst-within-tile, free columns = edge slots padded per-tile to the max
    degree.  Segment softmax -> free-dim reduce; message aggregation -> PSUM
    accumulation with an identity-matmul per slot column.  No scatter.
  * Per-edge source rows ([xw bf16 x128 | sc_s f32 x4], 512B-stride table,
    272B payload) are fetched with dma_gather from an AllGather-replicated
    DRAM table.  int16 gather indices only reach 32767 rows, so the table is
    addressed through two overlapping windows (A: rows [0, 32768), B: rows
    [17408, 50176)); each dst's edges are split into window-A and window-B
    slot sub-grids (balanced via nodes reachable from both windows).
  * Self-loops (fill='mean') are handled out-of-band in node space; since the
    reference's segment-max subtraction cancels exactly, exp terms move
    outside the segment sum, and normalization divides the aggregate.
"""

import sys
sys.path.insert(0, '/opt/trn_rl_repo')

import inspect
import textwrap

import numpy as np
import ml_dtypes

import concourse.bass as bass
import concourse.bacc as bacc
import concourse.tile as tile
import concourse.mybir as mybir

bf16 = ml_dtypes.bfloat16
F32 = mybir.dt.float32
BF16 = mybir.dt.bfloat16
I16 = mybir.dt.int16

# problem dims
N, E, F_IN, ED = 50000, 800000, 64, 16
H, C = 4, 32
HC = H * C
B, A = 64, 8
NEG_SLOPE = 0.2
NCORE = 8
NLOC = 6272
NT = 49
NTOT = NCORE * NLOC          # 50176
STRIDE = 256                 # bf16 elems per table row (512 B)
ROWE = 136                   # gathered elems per row (272 B)
WINB = 17408
WINA_MAX = 32767
POISON = -1.0e38
GBUDGET = 32                 # max slot columns (A+B) per gather group
POISON_A = 6271              # abs row, inside window A
POISON_B = 3 * NLOC + 6271   # abs row 25087, inside window B


def _patch_dma_gather():
    """Relax the elem_size_bytes % 256 assert (transpose-only restriction; the
    non-transpose HBM path takes arbitrary payload length, only the row stride
    must be a multiple of 256B)."""
    if getattr(bass.BassGpSimd.dma_gather, "_gat_patched", False):
        return
    src = textwrap.dedent(inspect.getsource(bass.BassGpSimd.dma_gather))
    needle = (
        "    assert (\n"
        "        elem_size_bytes > 0 and elem_size_bytes % 256 == 0\n"
        "    )  # transpose restriction\n"
    )
    assert needle in src, "dma_gather source changed; patch needs update"
    src = src.replace(
        needle,
        "    assert elem_size_bytes > 0\n"
        "    if transpose:\n"
        "        assert elem_size_bytes % 256 == 0\n",
    )
    ns = vars(bass).copy()
    exec(compile(src, "<patched dma_gather>", "exec"), ns)
    fn = ns["dma_gather"]
    fn._gat_patched = True
    bass.BassGpSimd.dma_gather = fn


# ===================================================================== prep
def _prep(inputs):
    x = np.asarray(inputs["x"], np.float32)
    edge_attr = np.asarray(inputs["edge_attr"], np.float32)
    edge_index = np.asarray(inputs["edge_index"]).astype(np.int64)
    batch = np.asarray(inputs["batch"]).astype(np.int64)
    src, dst = edge_index[0], edge_index[1]

    deg = np.bincount(dst, minlength=N)
    odeg = np.bincount(src, minlength=N)

    # node -> core; put high out-degree nodes into cores whose table blocks
    # fall in the shared window region (cores 3,4,2,5 cover rows ~12.5K-37.6K)
    order = np.argsort(-odeg, kind="stable")
    owner = np.empty(N, np.int64)
    for i, c in enumerate([3, 4, 2, 5, 1, 6, 0, 7]):
        owner[order[i * 6250:(i + 1) * 6250]] = c

    local = np.empty(N, np.int64)
    nodes_of = []
    for c in range(NCORE):
        mine = np.where(owner == c)[0]
        mine = mine[np.argsort(-deg[mine], kind="stable")]
        local[mine] = np.arange(6250)
        nodes_of.append(mine)
    pos = owner * NLOC + local

    # per-core, per-dst edge lists split into windows A/B (balanced)
    ecore = owner[dst]
    eloc = local[dst]
    RA = np.zeros(NT, np.int64)
    RB = np.zeros(NT, np.int64)
    core_lists = []
    for c in range(NCORE):
        sel = np.where(ecore == c)[0]
        d_loc = eloc[sel]
        ord2 = np.argsort(d_loc, kind="stable")
        sel = sel[ord2]
        d_loc = d_loc[ord2]
        spos = pos[src[sel]]
        okA = spos <= WINA_MAX
        okB = spos >= WINB
        bounds = np.searchsorted(d_loc, np.arange(6251))
        listsA = [None] * NLOC
        listsB = [None] * NLOC
        for ln in range(6250):
            lo, hi = bounds[ln], bounds[ln + 1]
            ea, eb = [], []
            if lo < hi:
                free = []
                for k in range(lo, hi):
                    if okA[k] and okB[k]:
                        free.append(sel[k])
                    elif okA[k]:
                        ea.append(sel[k])
                    else:
                        eb.append(sel[k])
                for e in free:
                    (ea if len(ea) <= len(eb) else eb).append(e)
            listsA[ln] = ea
            listsB[ln] = eb
            t = ln // 128
            RA[t] = max(RA[t], len(ea))
            RB[t] = max(RB[t], len(eb))
        core_lists.append((listsA, listsB))
    RA = np.maximum(RA, 1)
    RB = np.maximum(RB, 1)

    # gather groups: consecutive tiles, sum(RA+RB) <= GBUDGET
    groups = []
    cur, cwa, cwb = [], 0, 0
    for t in range(NT):
        if cur and cwa + cwb + RA[t] + RB[t] > GBUDGET:
            groups.append((cur, cwa, cwb))
            cur, cwa, cwb = [], 0, 0
        cur.append(t)
        cwa += int(RA[t])
        cwb += int(RB[t])
    groups.append((cur, cwa, cwb))

    # combined column layout: per group: [A slots of tiles][B slots of tiles]
    colA = np.zeros(NT, np.int64)
    colB = np.zeros(NT, np.int64)
    gstart = []
    w = 0
    for tiles, cwa, cwb in groups:
        gstart.append(w)
        for t in tiles:
            colA[t] = w
            w += int(RA[t])
        for t in tiles:
            colB[t] = w
            w += int(RB[t])
    WTOT = w
    W8 = (WTOT + 7) // 8

    GW = max(cwa + cwb for _, cwa, cwb in groups)
    layout = dict(RA=RA, RB=RB, WTOT=WTOT, W8=W8, colA=colA, colB=colB,
                  groups=groups, gstart=gstart, GW=GW)

    gcnt = np.bincount(batch, minlength=B).astype(np.float32)

    in_maps = []
    for c in range(NCORE):
        listsA, listsB = core_lists[c]
        gidx = np.zeros((128, WTOT), np.int64)
        attr_rect = np.zeros((128, W8 * 8, ED), np.float32)
        for ln in range(NLOC):
            t, j = ln // 128, ln % 128
            ea = listsA[ln] if ln < 6250 else []
            eb = listsB[ln] if ln < 6250 else []
            ca, cb = colA[t], colB[t]
            for r in range(RA[t]):
                if r < len(ea):
                    gidx[j, ca + r] = pos[src[ea[r]]]
                    attr_rect[j, ca + r] = edge_attr[ea[r]]
                else:
                    gidx[j, ca + r] = POISON_A
            for r in range(RB[t]):
                if r < len(eb):
                    gidx[j, cb + r] = pos[src[eb[r]]] - WINB
                    attr_rect[j, cb + r] = edge_attr[eb[r]]
                else:
                    gidx[j, cb + r] = POISON_B - WINB
        assert 0 <= gidx.min() and gidx.max() <= 32767
        gidx = gidx.astype(np.int16)

        # wrapped idx layout: position i=(col-c0)*128+j -> idx16[j%16, col*8+j//16]
        jj = np.arange(128)
        gidxw = np.zeros((16, WTOT * 8), np.int16)
        cols8 = (np.arange(WTOT)[None, :] * 8 + (jj // 16)[:, None])  # [128, WTOT]
        gidxw[(jj % 16)[:, None], cols8] = gidx
        gidxw = np.tile(gidxw, (8, 1))

        # attr8[g, wj*ED+cc, j] = attr_rect[j, 8g+wj, cc]
        a4 = attr_rect.reshape(128, W8, 8, ED)
        attr8 = np.ascontiguousarray(
            a4.transpose(1, 2, 3, 0).reshape(W8, 8 * ED, ED and 128)).astype(bf16)

        xblk = np.zeros((128, NT, F_IN), np.float32)
        rcnt = np.zeros((128, NT), np.float32)
        pmat = np.zeros((128, NT, B), np.float32)
        mine = nodes_of[c]
        for ln in range(6250):
            t, j = ln // 128, ln % 128
            n = mine[ln]
            xblk[j, t] = x[n]
            rcnt[j, t] = 1.0 / max(deg[n], 1.0)
            pmat[j, t, batch[n]] = 1.0 / max(gcnt[batch[n]], 1.0)
        rcnt[(np.arange(6250, NLOC) % 128), (np.arange(6250, NLOC) // 128)] = 1.0

        in_maps.append({"gidx": gidxw, "attr8": attr8, "xblk": xblk,
                        "rcnt": rcnt, "pmat": pmat})

    # weights (replicated)
    wts = {}
    q8s = []
    for li, (Wk, Wek, ask, adk, aek, bk) in enumerate(
            [("W1", "We1", "as1", "ad1", "ae1", "b1"),
             ("W2", "We2", "as2", "ad2", "ae2", "b2"),
             ("W3", "We3", "as3", "ad3", "ae3", "b3")]):
        Wm = np.asarray(inputs[Wk], np.float32)
        Wem = np.asarray(inputs[Wek], np.float32)
        a_s = np.asarray(inputs[ask], np.float32)
        a_d = np.asarray(inputs[adk], np.float32)
        a_e = np.asarray(inputs[aek], np.float32)
        bv = np.asarray(inputs[bk], np.float32)
        wts[f"w{li+1}"] = Wm.astype(bf16)
        asdb = np.zeros((HC, 8), np.float32)
        for h in range(H):
            asdb[h * C:(h + 1) * C, h] = a_s[h]
            asdb[h * C:(h + 1) * C, 4 + h] = a_d[h]
        wts[f"asdb{li+1}"] = asdb
        Q = np.zeros((ED, H), np.float32)
        for h in range(H):
            Q[:, h] = Wem[:, h * C:(h + 1) * C] @ a_e[h]
        q8 = np.zeros((128, 32), np.float32)
        for wj in range(8):
            q8[wj * ED:(wj + 1) * ED, wj * 4:(wj + 1) * 4] = Q
        q8s.append(q8)
        wts[f"bias{li+1}"] = bv.reshape(1, HC)
    wts["qblk"] = np.concatenate(q8s, axis=1).astype(bf16)
    wts["wl"] = np.asarray(inputs["Wl"], np.float32)
    wts["blv"] = np.asarray(inputs["bl"], np.float32).reshape(A, 1)
    wts["ident"] = np.eye(128, dtype=np.float32)
    wts["poisblk"] = np.full((NLOC - 6250, 4), POISON, np.float32).view(bf16)
    for m in in_maps:
        m.update(wts)
    return in_maps, layout


# ==================================================================== build
def build(layout):
    _patch_dma_gather()
    RA, RB = layout["RA"], layout["RB"]
    WTOT, W8 = layout["WTOT"], layout["W8"]
    colA, colB = layout["colA"], layout["colB"]
    groups, gstart = layout["groups"], layout["gstart"]
    GW = layout["GW"]

    nc = bacc.Bacc("TRN2", target_bir_lowering=False, debug=False,
                   num_devices=NCORE)

    gidx_in = nc.dram_tensor("gidx", [128, WTOT * 8], I16, kind="ExternalInput")
    attr8_in = nc.dram_tensor("attr8", [W8, 128, 128], BF16, kind="ExternalInput")
    xblk_in = nc.dram_tensor("xblk", [128, NT, F_IN], F32, kind="ExternalInput")
    rcnt_in = nc.dram_tensor("rcnt", [128, NT], F32, kind="ExternalInput")
    pmat_in = nc.dram_tensor("pmat", [128, NT, B], F32, kind="ExternalInput")
    w_in = {1: nc.dram_tensor("w1", [F_IN, HC], BF16, kind="ExternalInput"),
            2: nc.dram_tensor("w2", [HC, HC], BF16, kind="ExternalInput"),
            3: nc.dram_tensor("w3", [HC, HC], BF16, kind="ExternalInput")}
    asdb_in = {l: nc.dram_tensor(f"asdb{l}", [HC, 8], F32, kind="ExternalInput")
               for l in (1, 2, 3)}
    bias_in = {l: nc.dram_tensor(f"bias{l}", [1, HC], F32, kind="ExternalInput")
               for l in (1, 2, 3)}
    qblk_in = nc.dram_tensor("qblk", [128, 96], BF16, kind="ExternalInput")
    wl_in = nc.dram_tensor("wl", [HC, A], F32, kind="ExternalInput")
    blv_in = nc.dram_tensor("blv", [A, 1], F32, kind="ExternalInput")
    ident_in = nc.dram_tensor("ident", [128, 128], F32, kind="ExternalInput")
    pois_in = nc.dram_tensor("poisblk", [NLOC - 6250, 8], BF16, kind="ExternalInput")
    out_t = nc.dram_tensor("out", [A, B], F32, kind="ExternalOutput")

    blk = nc.dram_tensor("blk", [NLOC, STRIDE], BF16)
    tblS = nc.dram_tensor("tblS", [NTOT, STRIDE], BF16, addr_space="Shared")
    pool_in = nc.dram_tensor("pool_in", [HC, B], F32)
    pool_sh = nc.dram_tensor("pool_sh", [HC, B], F32, addr_space="Shared")

    tblap = tblS.ap()
    winA = tblap[:, :ROWE]
    winB = tblap[WINB:, :ROWE]
    rg = [list(range(NCORE))]

    with tile.TileContext(nc) as tc:
        with (
            tc.tile_pool(name="const", bufs=1) as cpool,
            tc.tile_pool(name="sb", bufs=3) as sb,
            tc.tile_pool(name="sclp", bufs=2) as sclp,
            tc.tile_pool(name="gp", bufs=2) as gp,
            tc.tile_pool(name="pp", bufs=1) as pp,
            tc.tile_pool(name="np2", bufs=2) as np2,
            tc.tile_pool(name="np1", bufs=1) as np1,
            tc.tile_pool(name="psA", bufs=2, space="PSUM") as psA,
            tc.tile_pool(name="psB", bufs=2, space="PSUM") as psB,
            tc.tile_pool(name="psC", bufs=2, space="PSUM") as psC,
            tc.tile_pool(name="psD", bufs=1, space="PSUM") as psD,
            tc.tile_pool(name="psE", bufs=1, space="PSUM") as psE,
        ):
            identf = cpool.tile([128, 128], F32)
            nc.sync.dma_start(identf[:], ident_in.ap())
            identb = cpool.tile([128, 128], BF16)
            nc.vector.tensor_copy(identb[:], identf[:])
            gidx = cpool.tile([128, WTOT * 8], I16)
            nc.sync.dma_start(gidx[:], gidx_in.ap())
            rcnt = cpool.tile([128, NT], F32)
            nc.sync.dma_start(rcnt[:], rcnt_in.ap())
            qblk = cpool.tile([128, 96], BF16)
            nc.sync.dma_start(qblk[:], qblk_in.ap())
            wts = {}
            for l in (1, 2, 3):
                wt = cpool.tile([F_IN if l == 1 else HC, HC], BF16, tag=f"w{l}")
                nc.sync.dma_start(wt[:], w_in[l].ap())
                ab = cpool.tile([HC, 8], F32, tag=f"asdb{l}")
                nc.sync.dma_start(ab[:], asdb_in[l].ap())
                bt = cpool.tile([1, HC], F32, tag=f"bias{l}")
                nc.sync.dma_start(bt[:], bias_in[l].ap())
                wts[l] = (wt, ab, bt)
            ones1 = cpool.tile([1, 128], F32)
            nc.gpsimd.memset(ones1[:], 1.0)
            btf = {}
            for l in (1, 2, 3):
                bp = psC.tile([128, HC], F32, tag="ps2", name="bp")
                nc.tensor.matmul(bp[:], lhsT=ones1[:], rhs=wts[l][2][:],
                                 start=True, stop=True)
                btx = cpool.tile([128, HC], F32, tag=f"btf{l}", name="btx")
                nc.vector.tensor_copy(btx[:], bp[:])
                btf[l] = btx
            wl = cpool.tile([HC, A], F32)
            nc.sync.dma_start(wl[:], wl_in.ap())
            blv = cpool.tile([A, 1], F32)
            nc.sync.dma_start(blv[:], blv_in.ap())

            # sc_e for the 3 layers in the combined slot layout
            sce = [pp.tile([128, W8 * 8, 4], BF16, tag=f"sce{l}",
                           name=f"sce{l}") for l in (1, 2, 3)]
            for g in range(W8):
                a8 = sb.tile([128, 128], BF16, tag="attr8")
                nc.sync.dma_start(a8[:], attr8_in.ap()[g])
                pse = psB.tile([128, 96], F32, tag="ps1")
                nc.tensor.matmul(pse[:], lhsT=a8[:], rhs=qblk[:], start=True,
                                 stop=True)
                for li in range(3):
                    nc.scalar.copy(
                        sce[li][:, g * 8:(g + 1) * 8, :],
                        pse[:, li * 32:(li + 1) * 32]
                        .rearrange("p (w h) -> p w h", h=4))

            def node_phase(l, h_of, combined, sc_sd):
                wt, ab, _ = wts[l]
                F = F_IN if l == 1 else HC
                for t in range(NT):
                    hT = psB.tile([F, 128], F32, tag="ps1")
                    nc.tensor.transpose(hT[:], h_of(t), identf[:])
                    hTs = sb.tile([F, 128], BF16, tag="hTs")
                    nc.scalar.copy(hTs[:], hT[:])
                    xwT = psC.tile([128, 128], F32, tag="ps2")
                    nc.tensor.matmul(xwT[:], lhsT=wt[:], rhs=hTs[:],
                                     start=True, stop=True)
                    xwTs = sb.tile([128, 128], F32, tag="xwTs")
                    nc.vector.tensor_copy(xwTs[:], xwT[:])
                    scp = psD.tile([128, 8], F32, tag="ps3")
                    nc.tensor.matmul(scp[:], lhsT=xwTs[:], rhs=ab[:],
                                     start=True, stop=True)
                    nc.vector.tensor_copy(sc_sd[:, t, :], scp[:])
                    xwN = psE.tile([128, 128], F32, tag="ps4")
                    nc.tensor.transpose(xwN[:], xwTs[:], identf[:])
                    nc.scalar.copy(combined[:, t, 0:128], xwN[:])
                    nc.vector.tensor_copy(
                        combined[:, t, 128:136].bitcast(F32), scp[:, 0:4])
                nc.sync.dma_start(
                    blk.ap()[:, :ROWE].rearrange("(t j) e -> j t e", j=128),
                    combined[:],
                )
                nc.sync.dma_start(blk.ap()[6250:NLOC, 128:136],
                                  pois_in.ap())
                nc.gpsimd.collective_compute(
                    "AllGather", mybir.AluOpType.bypass, replica_groups=rg,
                    ins=[blk.ap()], outs=[tblS.ap()],
                )

            comb0 = np2.tile([128, NT, ROWE], BF16, tag="comb")
            scsd0 = np2.tile([128, NT, 8], F32, tag="scsd")
            combined = {0: comb0, 1: None}
            sc_sd = {0: scsd0, 1: None}

            xw0 = None

            def x_of(t):
                xt = sb.tile([128, F_IN], F32, tag="xt")
                nc.sync.dma_start(xt[:], xblk_in.ap()[:, t, :])
                return xt[:]

            node_phase(1, x_of, combined[0], sc_sd[0])

            for l in (1, 2, 3):
                cur = combined[(l - 1) % 2]
                cur_sc = sc_sd[(l - 1) % 2]
                expc = np1.tile([128, WTOT, 4], F32, tag="expc")
                hbuf = np1.tile([128, NT, HC], F32, tag="hbuf")
                _, _, bt = wts[l]

                for gi, (tiles, cwa, cwb) in enumerate(groups):
                    w0 = gstart[gi]
                    gt = gp.tile([128, GW, ROWE], BF16, tag="g")
                    # window-A gather covers cols [w0, w0+cwa); B the rest
                    nc.gpsimd.dma_gather(
                        out_ap=gt[:, :cwa, :], in_ap=winA,
                        idxs_ap=gidx[:, w0 * 8:(w0 + cwa) * 8],
                        num_idxs=cwa * 128, num_idxs_reg=cwa * 128,
                        elem_size=ROWE, elem_step=STRIDE, single_packet=False)
                    nc.gpsimd.dma_gather(
                        out_ap=gt[:, cwa:cwa + cwb, :], in_ap=winB,
                        idxs_ap=gidx[:, (w0 + cwa) * 8:(w0 + cwa + cwb) * 8],
                        num_idxs=cwb * 128, num_idxs_reg=cwb * 128,
                        elem_size=ROWE, elem_step=STRIDE, single_packet=False)

                    def edge_block(t, wc, rt, agg, first, last, dnm):
                        """Process slot columns [wc, wc+rt) (combined space)
                        for tile t: alpha, scaled messages, psum accumulate,
                        and add the exp-sum into dnm."""
                        co = wc - w0
                        gsl = gt[:, co:co + rt, :]
                        pa = sb.tile([128, GW, 4], F32, tag="pa",
                                     name="pa")
                        nc.vector.tensor_add(
                            pa[:, :rt, :],
                            gsl[:, :, 128:136].bitcast(F32),
                            sce[l - 1][:, wc:wc + rt, :])
                        nc.vector.tensor_add(
                            pa[:, :rt, :], pa[:, :rt, :],
                            cur_sc[:, t, 4:8].unsqueeze(1)
                            .to_broadcast([128, rt, 4]))
                        pb = sb.tile([128, GW, 4], F32, tag="pb",
                                     name="pb")
                        nc.vector.tensor_scalar(
                            pb[:, :rt, :], pa[:, :rt, :], NEG_SLOPE, None,
                            mybir.AluOpType.mult)
                        nc.vector.tensor_tensor(
                            pa[:, :rt, :], pa[:, :rt, :], pb[:, :rt, :],
                            mybir.AluOpType.max)
                        nc.scalar.activation(
                            expc[:, wc:wc + rt, :], pa[:, :rt, :],
                            mybir.ActivationFunctionType.Exp)
                        expb = sb.tile([128, GW, 4], BF16, tag="expb",
                                       name="expb")
                        nc.vector.tensor_copy(expb[:, :rt, :],
                                              expc[:, wc:wc + rt, :])
                        scl = sclp.tile([128, GW, HC], BF16, tag="scl",
                                        name="scl")
                        nc.vector.tensor_tensor(
                            scl[:, :rt, :]
                            .rearrange("p r (h c) -> p r h c", h=4),
                            gsl[:, :, 0:128]
                            .rearrange("p r (h c) -> p r h c", h=4),
                            expb[:, :rt, :].unsqueeze(3)
                            .to_broadcast([128, rt, 4, C]),
                            mybir.AluOpType.mult)
                        for r in range(rt):
                            nc.tensor.matmul(
                                agg[:], lhsT=identb[:], rhs=scl[:, r, :],
                                start=(first and r == 0),
                                stop=(last and r == rt - 1))
                        red = sb.tile([128, 4], F32, tag="red", name="red")
                        nc.vector.tensor_reduce(
                            red[:],
                            expc[:, wc:wc + rt, :].rearrange("p r h -> p h r"),
                            axis=mybir.AxisListType.X, op=mybir.AluOpType.add)
                        if first:
                            nc.vector.tensor_copy(dnm[:], red[:])
                        else:
                            nc.vector.tensor_add(dnm[:], dnm[:], red[:])

                    for t in tiles:
                        ra, rb = int(RA[t]), int(RB[t])
                        agg = psA.tile([128, HC], F32, tag="agg", name="agg")
                        dnm = sb.tile([128, 4], F32, tag="dnm", name="dnm")
                        edge_block(t, int(colA[t]), ra, agg, True, False, dnm)
                        edge_block(t, int(colB[t]), rb, agg, False, True, dnm)
                        # self-loop sc_e_loop = (segsum sce_A + segsum sce_B)/cnt
                        sl = sb.tile([128, 4], F32, tag="sl", name="sl")
                        nc.vector.tensor_reduce(
                            sl[:],
                            sce[l - 1][:, int(colA[t]):int(colA[t]) + ra, :]
                            .rearrange("p r h -> p h r"),
                            axis=mybir.AxisListType.X, op=mybir.AluOpType.add)
                        sl2 = sb.tile([128, 4], F32, tag="sl2", name="sl2")
                        nc.vector.tensor_reduce(
                            sl2[:],
                            sce[l - 1][:, int(colB[t]):int(colB[t]) + rb, :]
                            .rearrange("p r h -> p h r"),
                            axis=mybir.AxisListType.X, op=mybir.AluOpType.add)
                        nc.vector.tensor_add(sl[:], sl[:], sl2[:])
                        nc.vector.tensor_tensor(
                            sl[:], sl[:],
                            rcnt[:, t:t + 1].to_broadcast([128, 4]),
                            mybir.AluOpType.mult)
                        nc.vector.tensor_add(sl[:], sl[:], cur_sc[:, t, 0:4])
                        nc.vector.tensor_add(sl[:], sl[:], cur_sc[:, t, 4:8])
                        nc.vector.tensor_scalar(
                            sl2[:], sl[:], NEG_SLOPE, None,
                            mybir.AluOpType.mult)
                        nc.vector.tensor_tensor(sl[:], sl[:], sl2[:],
                                                mybir.AluOpType.max)
                        nc.scalar.activation(
                            sl[:], sl[:], mybir.ActivationFunctionType.Exp)
                        nc.vector.tensor_add(dnm[:], dnm[:], sl[:])
                        nc.vector.tensor_scalar(
                            dnm[:], dnm[:], 1e-16, None, mybir.AluOpType.add)
                        rec = sb.tile([128, 4], F32, tag="rec", name="rec")
                        nc.vector.reciprocal(rec[:], dnm[:])
                        # h = relu((agg + exp_loop*xw_local) * rec + bias)
                        lt = sb.tile([128, HC], F32, tag="lt", name="lt")
                        nc.vector.tensor_tensor(
                            lt[:].rearrange("p (h c) -> p h c", h=4),
                            cur[:, t, 0:128]
                            .rearrange("p (h c) -> p h c", h=4),
                            sl[:].unsqueeze(2).to_broadcast([128, 4, C]),
                            mybir.AluOpType.mult)
                        nc.vector.tensor_add(lt[:], lt[:], agg[:])
                        nc.vector.tensor_tensor(
                            lt[:].rearrange("p (h c) -> p h c", h=4),
                            lt[:].rearrange("p (h c) -> p h c", h=4),
                            rec[:].unsqueeze(2).to_broadcast([128, 4, C]),
                            mybir.AluOpType.mult)
                        nc.vector.tensor_add(lt[:], lt[:], btf[l][:])
                        nc.vector.tensor_scalar(
                            hbuf[:, t, :], lt[:], 0.0, None,
                            mybir.AluOpType.max)

                if l < 3:
                    combN = np2.tile([128, NT, ROWE], BF16, tag="comb")
                    scsdN = np2.tile([128, NT, 8], F32, tag="scsd")
                    combined[l % 2] = combN
                    sc_sd[l % 2] = scsdN
                    node_phase(l + 1, lambda t: hbuf[:, t, :], combN, scsdN)
                else:
                    pl = psB.tile([HC, B], F32, tag="ps1")
                    for t in range(NT):
                        pm = sb.tile([128, B], F32, tag="pm")
                        nc.sync.dma_start(pm[:], pmat_in.ap()[:, t, :])
                        nc.tensor.matmul(pl[:], lhsT=hbuf[:, t, :], rhs=pm[:],
                                         start=(t == 0), stop=(t == NT - 1))
                    pls = sb.tile([HC, B], F32, tag="pls")
                    nc.vector.tensor_copy(pls[:], pl[:])
                    nc.sync.dma_start(pool_in.ap(), pls[:])
                    nc.gpsimd.collective_compute(
                        "AllReduce", mybir.AluOpType.add, replica_groups=rg,
                        ins=[pool_in.ap()], outs=[pool_sh.ap()])
                    plr = sb.tile([HC, B], F32, tag="plr")
                    nc.sync.dma_start(plr[:], pool_sh.ap())
                    zt = psC.tile([A, B], F32, tag="ps2")
                    nc.tensor.matmul(zt[:], lhsT=wl[:], rhs=plr[:],
                                     start=True, stop=True)
                    ot = sb.tile([A, B], F32, tag="ot")
                    nc.scalar.activation(
                        ot[:], zt[:], mybir.ActivationFunctionType.Tanh,
                        bias=blv[:])
                    nc.sync.dma_start(out_t.ap(), ot[:])
    nc.compile()
    return nc


# ================================================================== entry
_CACHE = {}


def _get_nc(layout):
    key = (layout["WTOT"], layout["GW"], tuple(layout["RA"]), tuple(layout["RB"]))
    if key not in _CACHE:
        _CACHE[key] = build(layout)
    return _CACHE[key]


def kernel(**inputs):
    in_maps, layout = _prep(inputs)
    nc = _get_nc(layout)
    from concourse import bass2jax
    results = bass2jax.run_bass_via_pjrt(nc, in_maps, n_cores=NCORE)
    return np.ascontiguousarray(np.asarray(results[0]["out"], np.float32).T)

